# revision 23
# baseline (speedup 1.0000x reference)
"""Sliding-window causal self-attention (GQA + RoPE + tanh softcap) on 8 trn2 cores.

Sharding: core i = (b, g) with b = i // 4, g = i % 4.
Each core computes, for its batch b and kv-group g (4 q heads, 1 kv head):
    qkv projection (o-slice), RoPE, sliding-window attention, and the proj
    contribution of its o-slice:  out_partial[t, c] = sum_{o in slice} y[t,o] Wproj[c,o].
The host sums the 4 partials per batch (the "all-reduce after proj" done host-side).

All matmuls run as fp32r (full PE rate at N=512 when warm). Layouts avoid
on-device transposes except v (PE-transpose via identity):
    xT      [C, T]   (host-transposed x[b])
    wqkvT   [C, 768] (host-transposed o-slice of Wqkv; o order: q0..q3, k, v)
    wprojT  [512, C] (host-transposed o-slice columns of Wproj)
    qT/kT   [d, t]   from  wT.T @ xT  (d on partitions -> scoresT = kT_tile.T @ qT)
    scoresT [j, i]   j (keys) on partitions, i (queries) on free axis
    P = exp(softcap(scores)) stays [j, i]; yT = v_tile.T @ P accumulates [d, i]
    rowsum via ones-matmul [1, i]; normalize = PE-broadcast recip + TT mult.

Block pipeline (emission order): L0 Q0 L1 A0 Q1 P0 L2 A1 Q2 P1 L3 A2 Q3 P2 A3 P3
so the PE always has qkv/proj matmul work while attention tails (rowsum
reciprocal chains) drain on DVE/DMA. Each head's normalize is split:
  stageA (DVE copy of psR + reshape-DMA + recip[128,4] + DMA back) emitted
  right after the head's matmuls; stageC (psB broadcast MM + rrf copy + yT
  mult) deferred past the next head's matmuls so the PE never waits on it.
"""

import math

import numpy as np

import concourse.bass as bass
import concourse.mybir as mybir
import concourse.tile as tile
from concourse.bass_utils import run_bass_kernel_spmd
from concourse.masks import make_identity

B, T, C = 2, 2048, 2048
N_HEAD, N_GROUPS, HEAD_SIZE = 16, 4, 128
SW = 1024
SOFTCAP = 50.0
QBLK = 512
NQB = T // QBLK          # 4 q-blocks
NKT = T // 128           # 16 key tiles
O_SLICE = 768            # 4 q heads + k + v  (128 each)
F32 = mybir.dt.float32
F32R = mybir.dt.float32r
MASKW = 896              # staircase mask width: x = li - 128r, x in [-384, 512)


def _window(qt):
    """Key-tile list for q-block qt: (kt, mask) with mask None | ('D', r) | ('T', r)."""
    wl = []
    if qt >= 2:
        for r in range(4):
            wl.append((4 * qt - 8 + r, ("T", r)))
    for kt in range(max(0, 4 * qt - 4), 4 * qt):
        wl.append((kt, None))
    for r in range(4):
        wl.append((4 * qt + r, ("D", r)))
    return wl


def _emit(tc, ctx):
    nc = tc.nc
    xT = nc.declare_dram_parameter("xT", [C, T], F32R, isOutput=False)
    wqkvT = nc.declare_dram_parameter("wqkvT", [C, O_SLICE], F32R, isOutput=False)
    wprojT = nc.declare_dram_parameter("wprojT", [512, C], F32R, isOutput=False)
    cosT = nc.declare_dram_parameter("cosT", [HEAD_SIZE, T], F32, isOutput=False)
    sinS = nc.declare_dram_parameter("sinS", [HEAD_SIZE, T], F32, isOutput=False)
    maskD = nc.declare_dram_parameter("maskD", [128, MASKW], F32R, isOutput=False)
    maskTl = nc.declare_dram_parameter("maskTl", [128, MASKW], F32R, isOutput=False)
    out = nc.declare_dram_parameter("out", [T, C], F32, isOutput=True)
    rscratch = nc.dram_tensor("rscratch", [NQB * 4, QBLK], F32)

    scale1 = 1.0 / (SOFTCAP * math.sqrt(HEAD_SIZE))

    consts = ctx.enter_context(tc.tile_pool(name="consts", bufs=1))
    xt_pool = ctx.enter_context(tc.tile_pool(name="xt", bufs=16))
    cs_pool = ctx.enter_context(tc.tile_pool(name="cs", bufs=2))
    rope_pool = ctx.enter_context(tc.tile_pool(name="rope", bufs=3))
    p_pool = ctx.enter_context(tc.tile_pool(name="pp", bufs=5))
    o_pool = ctx.enter_context(tc.tile_pool(name="op", bufs=3))
    r_pool = ctx.enter_context(tc.tile_pool(name="rp", bufs=2))
    ps = ctx.enter_context(tc.tile_pool(name="ps", space="PSUM", bufs=2))

    # ---- resident constants (x block 0 interleaved for fast PE start) ----
    wq_sb = consts.tile([128, NKT, O_SLICE], F32R, name="wq_sb")
    x_tiles = {}  # (qt, k) -> tile
    for k in range(NKT):
        nc.sync.dma_start(out=wq_sb[:, k, :], in_=wqkvT[k * 128:(k + 1) * 128, :])
        x_t = xt_pool.tile([128, QBLK], F32R, name=f"x_0_{k}", tag="xt")
        nc.sync.dma_start(out=x_t, in_=xT[k * 128:(k + 1) * 128, 0:QBLK])
        x_tiles[(0, k)] = x_t
    cs_tiles = {}
    cos_b = cs_pool.tile([128, QBLK], F32, name="cos_0", tag="cos")
    nc.sync.dma_start(out=cos_b, in_=cosT[:, 0:QBLK])
    sin_b = cs_pool.tile([128, QBLK], F32, name="sin_0", tag="sin")
    nc.sync.dma_start(out=sin_b, in_=sinS[:, 0:QBLK])
    cs_tiles[0] = (cos_b, sin_b)

    wp_sb = consts.tile([128, 4, C], F32R, name="wp_sb")  # loaded after A0
    mD_sb = consts.tile([128, MASKW], F32R, name="mD_sb")
    nc.sync.dma_start(out=mD_sb, in_=maskD[:, :])
    mT_sb = consts.tile([128, MASKW], F32R, name="mT_sb")
    nc.sync.dma_start(out=mT_sb, in_=maskTl[:, :])
    # all-ones views carved out of the diag mask (f32r, DMA-fed):
    ones_col = mD_sb[:, MASKW - 1:MASKW]     # [128, 1]
    ones_row = mD_sb[0:1, 384:512]           # [1, 128]
    ident = consts.tile([128, 128], F32, name="ident")
    make_identity(nc, ident)
    # warm the ACT exp/tanh table set during the startup DMAs (first real
    # tanh would otherwise pay the ~2.7us ACT_TABLE_LOAD mid-pipeline)
    warmup = consts.tile([1, 1], F32, name="warmup")
    nc.scalar.activation(warmup, ident[0:1, 0:1],
                         mybir.ActivationFunctionType.Tanh)

    # persistent activations (written per block, sub-tile deps handle reuse)
    kT_sb = consts.tile([128, T], F32R, name="kT_sb")          # roped k, [d, t]
    v_sb = consts.tile([128, NKT, 128], F32R, name="v_sb")     # [t128, kt, d]
    qT_sb = consts.tile([128, 4, QBLK], F32R, name="qT_sb")    # roped q, [d, h, i]
    y_tiles = {}  # qt -> [128, 4, QBLK] tile, bufs=2 across blocks

    def emit_loads(qt):
        t0 = qt * QBLK
        for k in range(NKT):
            x_t = xt_pool.tile([128, QBLK], F32R, name=f"x_{qt}_{k}", tag="xt")
            nc.sync.dma_start(out=x_t, in_=xT[k * 128:(k + 1) * 128, t0:t0 + QBLK])
            x_tiles[(qt, k)] = x_t
        cos_b = cs_pool.tile([128, QBLK], F32, name=f"cos_{qt}", tag="cos")
        nc.sync.dma_start(out=cos_b, in_=cosT[:, t0:t0 + QBLK])
        sin_b = cs_pool.tile([128, QBLK], F32, name=f"sin_{qt}", tag="sin")
        nc.sync.dma_start(out=sin_b, in_=sinS[:, t0:t0 + QBLK])
        cs_tiles[qt] = (cos_b, sin_b)

    def emit_qkv_mtile(qt, m):
        t0 = qt * QBLK
        cos_b, sin_b = cs_tiles[qt]
        psA = ps.tile([128, QBLK], F32, name=f"psA_{qt}_{m}", tag="psA")
        for k in range(NKT):
            nc.tensor.matmul(
                psA,
                wq_sb[:, k, m * 128:(m + 1) * 128],
                x_tiles[(qt, k)],
                start=(k == 0),
                stop=(k == NKT - 1),
            )
        if m < 5:
            # RoPE: dest = x*cos + rot(x)*sin_signed ; rot via DMA half-swap
            x_sb = rope_pool.tile([128, QBLK], F32, name=f"xsb_{qt}_{m}", tag="xsb")
            nc.scalar.copy(x_sb, psA)
            rot = rope_pool.tile([128, QBLK], F32, name=f"rot_{qt}_{m}", tag="rot")
            nc.gpsimd.dma_start(out=rot[0:64, :], in_=x_sb[64:128, :])
            nc.gpsimd.dma_start(out=rot[64:128, :], in_=x_sb[0:64, :])
            dest = qT_sb[:, m, :] if m < 4 else kT_sb[:, t0:t0 + QBLK]
            nc.vector.tensor_mul(x_sb, x_sb, cos_b)
            nc.vector.tensor_mul(rot, rot, sin_b)
            nc.vector.tensor_add(dest, x_sb, rot)
        else:
            # v: transpose [d, t] -> [t, d] tiles via PE
            vt_sb = rope_pool.tile([128, QBLK], F32, name=f"vt_{qt}", tag="xsb")
            nc.scalar.copy(vt_sb, psA)
            for i in range(4):
                psT = ps.tile([128, 128], F32, name=f"psT_{qt}_{i}", tag="psT", bufs=1)
                nc.tensor.transpose(psT, vt_sb[:, i * 128:(i + 1) * 128], ident)
                nc.vector.tensor_copy(v_sb[:, qt * 4 + i, :], psT)

    def head_mms(qt, h, wl):
        """Scores/tanh-exp/mask/pv/rowsum matmul stream for one head.
        Scores are emitted one j-tile ahead of the pv/rowsum consumers so the
        PE never sits waiting on the ACT tanh+exp latency of the current tile.
        Returns the PSUM tiles needed by the normalize stages."""
        psY = ps.tile([128, QBLK], F32, name=f"psY_{qt}_{h}", tag="psY")
        psR = ps.tile([1, QBLK], F32, name=f"psR_{qt}_{h}", tag="psR", bufs=1)

        def emit_scores(idx):
            kt, mk = wl[idx]
            psS = ps.tile([128, QBLK], F32, name=f"psS_{qt}_{h}_{kt}", tag="psS")
            nc.tensor.matmul(
                psS, kT_sb[:, kt * 128:(kt + 1) * 128], qT_sb[:, h, :],
                start=True, stop=True,
            )
            p_t = p_pool.tile([128, QBLK], F32R, name=f"p_{qt}_{h}_{kt}", tag="p")
            nc.scalar.activation(
                p_t, psS, mybir.ActivationFunctionType.Tanh, scale=scale1
            )
            nc.scalar.activation(
                p_t, p_t, mybir.ActivationFunctionType.Exp, scale=SOFTCAP
            )
            if mk is not None:
                msk = mD_sb if mk[0] == "D" else mT_sb
                r = mk[1]
                nc.vector.tensor_mul(p_t, p_t, msk[:, 384 - 128 * r: 896 - 128 * r])
            return p_t

        pts = {0: emit_scores(0)}
        for idx, (kt, mk) in enumerate(wl):
            if idx + 1 < len(wl):
                pts[idx + 1] = emit_scores(idx + 1)
            p_t = pts.pop(idx)
            first, last = idx == 0, idx == len(wl) - 1
            nc.tensor.matmul(
                psY, v_sb[:, kt, :], p_t,
                start=first, stop=last, skip_group_check=True,
            )
            nc.tensor.matmul(
                psR, ones_col, p_t,
                start=first, stop=last, skip_group_check=True,
            )
        return psY, psR

    def norm_head(qt, h, psY, psR):
        """Free both PSUM accumulators fast with copies, then run the
        reciprocal + partition-broadcast + multiply entirely on DVE/DMA —
        the PE never participates. recip runs on a [128,4] reshape (DVE
        recip is ~6 cyc/elem/lane; [1,512] would serialize 3.3us)."""
        rs = r_pool.tile([1, QBLK], F32, name=f"rs_{qt}_{h}", tag="rs")
        nc.vector.tensor_copy(rs, psR)
        yun = r_pool.tile([128, QBLK], F32, name=f"yun_{qt}_{h}", tag="yun")
        nc.vector.tensor_copy(yun, psY)
        rs128 = r_pool.tile([128, 4], F32, name=f"rs128_{qt}_{h}", tag="rs128")
        in_lin = bass.AP(tensor=rs.tensor, offset=rs.offset,
                         ap=[list(rs.ap[0]), [1, QBLK]])
        nc.gpsimd.dma_start(out=rs128, in_=in_lin)
        rr128 = r_pool.tile([128, 4], F32, name=f"rr128_{qt}_{h}", tag="rr128")
        nc.vector.reciprocal(rr128, rs128)
        # bounce through DRAM to broadcast across partitions (stride-0 DRAM
        # read on the way back — the standard partition-broadcast pattern)
        slot = rscratch[qt * 4 + h, :]
        nc.gpsimd.dma_start(out=slot, in_=rr128)
        rrf = r_pool.tile([128, QBLK], F32, name=f"rrf_{qt}_{h}", tag="rrf")
        bcast = bass.AP(tensor=slot.tensor, offset=slot.offset,
                        ap=[[0, 128], list(slot.ap[-1])])
        nc.gpsimd.dma_start(out=rrf, in_=bcast)
        nc.vector.tensor_mul(y_tiles[qt][:, h, :], yun, rrf)

    def emit_proj_chunk(qt, mt):
        t0 = qt * QBLK
        yt = y_tiles[qt]
        for cn in range(4):
            psP = ps.tile([128, 512], F32, name=f"psP_{qt}_{mt}_{cn}", tag="psA")
            for kh in range(4):
                nc.tensor.matmul(
                    psP,
                    yt[:, kh, mt * 128:(mt + 1) * 128],
                    wp_sb[:, kh, cn * 512:(cn + 1) * 512],
                    start=(kh == 0),
                    stop=(kh == 3),
                )
            o_t = o_pool.tile([128, 512], F32, name=f"o_{qt}_{mt}_{cn}", tag="o")
            nc.vector.tensor_copy(o_t, psP)
            nc.sync.dma_start(
                out=out[t0 + mt * 128: t0 + (mt + 1) * 128,
                        cn * 512:(cn + 1) * 512],
                in_=o_t,
            )

    # ---- interleaved pipeline ----
    # m-tile fill schedule: the m-tile feeding head h is always emitted at
    # least one head earlier (rope latency hidden), k/v of the next block two
    # heads early. Proj chunks of block qt-1 fill the remaining head slots.
    def fills_for(qt, h):
        if h == 0:
            f = [(qt, 2)]
        elif h == 1:
            f = [(qt, 3)]
        elif h == 2:
            f = [(qt + 1, 4), (qt + 1, 5)]
        else:
            f = [(qt + 1, 0), (qt + 1, 1)]
        return [(q, m) for (q, m) in f if q < NQB]

    for q, m in [(0, 4), (0, 5), (0, 0), (0, 1)]:
        emit_qkv_mtile(q, m)
    for qt in range(NQB):
        if qt + 1 < NQB:
            emit_loads(qt + 1)
        wl = _window(qt)
        y_tiles[qt] = consts.tile([128, 4, QBLK], F32R,
                                  name=f"yT_{qt}", tag="yT", bufs=2)
        for h in range(4):
            psY, psR = head_mms(qt, h, wl)
            norm_head(qt, h, psY, psR)
            for q, m in fills_for(qt, h):
                emit_qkv_mtile(q, m)
            if qt >= 1:
                emit_proj_chunk(qt - 1, h)
        if qt == 0:
            for k in range(4):
                nc.sync.dma_start(out=wp_sb[:, k, :],
                                  in_=wprojT[k * 128:(k + 1) * 128, :])
    for mt in range(4):
        emit_proj_chunk(NQB - 1, mt)

_NC_CACHE = {}


def _build_nc():
    if "nc" not in _NC_CACHE:
        from contextlib import ExitStack

        from concourse import bacc

        nc = bacc.Bacc()
        with tile.TileContext(nc) as tc, ExitStack() as ctx:
            _emit(tc, ctx)
        nc.compile()
        _NC_CACHE["nc"] = nc
    return _NC_CACHE["nc"]


def _host_inputs(x, cos, sin, Wqkv, Wproj):
    """Build the 8 per-core input maps (sharding + layout transforms)."""
    cosT = np.ascontiguousarray(cos.T)                       # [128, T]
    sinT = sin.T
    sinS = np.concatenate([-sinT[:64], sinT[64:]], axis=0)   # sign-folded rotate-half
    sinS = np.ascontiguousarray(sinS)
    lj = np.arange(128)[:, None]
    xcol = np.arange(MASKW)[None, :] - 384                   # x = li - 128r
    maskD = (xcol >= lj).astype(np.float32)                  # diag: li - 128r >= lj
    maskTl = (xcol < lj).astype(np.float32)                  # tail: li - 128r <  lj
    q_sz = N_HEAD * HEAD_SIZE

    in_maps = []
    for core in range(8):
        b, g = core // 4, core % 4
        xTb = np.ascontiguousarray(x[b].T)                   # [C, T]
        wslice = np.concatenate(
            [
                Wqkv[512 * g: 512 * (g + 1)],                 # 4 q heads
                Wqkv[q_sz + 128 * g: q_sz + 128 * (g + 1)],   # k
                Wqkv[q_sz + 512 + 128 * g: q_sz + 512 + 128 * (g + 1)],  # v
            ],
            axis=0,
        )                                                     # [768, C]
        wqkvT = np.ascontiguousarray(wslice.T)                # [C, 768]
        wprojT = np.ascontiguousarray(Wproj[:, 512 * g: 512 * (g + 1)].T)  # [512, C]
        in_maps.append(
            {
                "xT": xTb,
                "wqkvT": wqkvT,
                "wprojT": wprojT,
                "cosT": cosT,
                "sinS": sinS,
                "maskD": maskD,
                "maskTl": maskTl,
            }
        )
    return in_maps


def kernel(x, cos, sin, Wqkv, Wproj, trace=False, tmpdir=None):
    x = np.asarray(x, dtype=np.float32)
    cos = np.asarray(cos, dtype=np.float32)
    sin = np.asarray(sin, dtype=np.float32)
    Wqkv = np.asarray(Wqkv, dtype=np.float32)
    Wproj = np.asarray(Wproj, dtype=np.float32)

    nc = _build_nc()
    in_maps = _host_inputs(x, cos, sin, Wqkv, Wproj)
    res = run_bass_kernel_spmd(nc, in_maps, list(range(8)), trace=trace, tmpdir=tmpdir)
    parts = [res.results[i]["out"] for i in range(8)]
    out = np.empty((B, T, C), dtype=np.float32)
    out[0] = parts[0] + parts[1] + parts[2] + parts[3]
    out[1] = parts[4] + parts[5] + parts[6] + parts[7]
    if trace:
        kernel.last_exec_time_ns = res.exec_time_ns
        kernel.last_results = res
    return out


# revision 25
# speedup vs baseline: 1.0447x; 1.0447x over previous
"""Sliding-window causal self-attention (GQA + RoPE + tanh softcap) on 8 trn2 cores.

Sharding: core i = (b, g) with b = i // 4, g = i % 4.
Each core computes, for its batch b and kv-group g (4 q heads, 1 kv head):
    qkv projection (o-slice), RoPE, sliding-window attention, and the proj
    contribution of its o-slice:  out_partial[t, c] = sum_{o in slice} y[t,o] Wproj[c,o].
The host sums the 4 partials per batch (the "all-reduce after proj" done host-side).

All matmuls run as fp32r (full PE rate at N=512 when warm). Layouts avoid
on-device transposes except v (PE-transpose via identity):
    xT      [C, T]   (host-transposed x[b])
    wqkvT   [C, 768] (host-transposed o-slice of Wqkv; o order: q0..q3, k, v)
    wprojT  [512, C] (host-transposed o-slice columns of Wproj)
    qT/kT   [d, t]   from  wT.T @ xT  (d on partitions -> scoresT = kT_tile.T @ qT)
    scoresT [j, i]   j (keys) on partitions, i (queries) on free axis
    P = exp(softcap(scores)) stays [j, i]; yT = v_tile.T @ P accumulates [d, i]
    rowsum via ones-matmul [1, i]; normalize = PE-broadcast recip + TT mult.

Block pipeline (emission order): L0 Q0 L1 A0 Q1 P0 L2 A1 Q2 P1 L3 A2 Q3 P2 A3 P3
so the PE always has qkv/proj matmul work while attention tails (rowsum
reciprocal chains) drain on DVE/DMA. Each head's normalize is split:
  stageA (DVE copy of psR + reshape-DMA + recip[128,4] + DMA back) emitted
  right after the head's matmuls; stageC (psB broadcast MM + rrf copy + yT
  mult) deferred past the next head's matmuls so the PE never waits on it.
"""

import math

import numpy as np

import concourse.bass as bass
import concourse.mybir as mybir
import concourse.tile as tile
from concourse.bass_utils import run_bass_kernel_spmd
from concourse.masks import make_identity

B, T, C = 2, 2048, 2048
N_HEAD, N_GROUPS, HEAD_SIZE = 16, 4, 128
SW = 1024
SOFTCAP = 50.0
QBLK = 512
NQB = T // QBLK          # 4 q-blocks
NKT = T // 128           # 16 key tiles
O_SLICE = 768            # 4 q heads + k + v  (128 each)
F32 = mybir.dt.float32
F32R = mybir.dt.float32r
MASKW = 896              # staircase mask width: x = li - 128r, x in [-384, 512)


def _window(qt):
    """Key-tile list for q-block qt: (kt, mask) with mask None | ('D', r) | ('T', r)."""
    wl = []
    if qt >= 2:
        for r in range(4):
            wl.append((4 * qt - 8 + r, ("T", r)))
    for kt in range(max(0, 4 * qt - 4), 4 * qt):
        wl.append((kt, None))
    for r in range(4):
        wl.append((4 * qt + r, ("D", r)))
    return wl


def _emit(tc, ctx):
    nc = tc.nc
    xT = nc.declare_dram_parameter("xT", [C, T], F32R, isOutput=False)
    wqkvT = nc.declare_dram_parameter("wqkvT", [C, O_SLICE], F32R, isOutput=False)
    wprojT = nc.declare_dram_parameter("wprojT", [512, C], F32R, isOutput=False)
    cosT = nc.declare_dram_parameter("cosT", [HEAD_SIZE, T], F32, isOutput=False)
    sinS = nc.declare_dram_parameter("sinS", [HEAD_SIZE, T], F32, isOutput=False)
    maskD = nc.declare_dram_parameter("maskD", [128, MASKW], F32R, isOutput=False)
    maskTl = nc.declare_dram_parameter("maskTl", [128, MASKW], F32R, isOutput=False)
    out = nc.declare_dram_parameter("out", [T, C], F32, isOutput=True)
    rscratch = nc.dram_tensor("rscratch", [NQB * 4, QBLK], F32)

    scale1 = 1.0 / (SOFTCAP * math.sqrt(HEAD_SIZE))

    consts = ctx.enter_context(tc.tile_pool(name="consts", bufs=1))
    xt_pool = ctx.enter_context(tc.tile_pool(name="xt", bufs=16))
    cs_pool = ctx.enter_context(tc.tile_pool(name="cs", bufs=2))
    rope_pool = ctx.enter_context(tc.tile_pool(name="rope", bufs=3))
    p_pool = ctx.enter_context(tc.tile_pool(name="pp", bufs=5))
    o_pool = ctx.enter_context(tc.tile_pool(name="op", bufs=3))
    r_pool = ctx.enter_context(tc.tile_pool(name="rp", bufs=2))
    ps = ctx.enter_context(tc.tile_pool(name="ps", space="PSUM", bufs=2))

    # ---- resident constants (x block 0 interleaved for fast PE start) ----
    wq_sb = consts.tile([128, NKT, O_SLICE], F32R, name="wq_sb")
    x_tiles = {}  # (qt, k) -> tile
    for k in range(NKT):
        nc.sync.dma_start(out=wq_sb[:, k, :], in_=wqkvT[k * 128:(k + 1) * 128, :])
        x_t = xt_pool.tile([128, QBLK], F32R, name=f"x_0_{k}", tag="xt")
        nc.sync.dma_start(out=x_t, in_=xT[k * 128:(k + 1) * 128, 0:QBLK])
        x_tiles[(0, k)] = x_t
    cs_tiles = {}
    cos_b = cs_pool.tile([128, QBLK], F32, name="cos_0", tag="cos")
    nc.sync.dma_start(out=cos_b, in_=cosT[:, 0:QBLK])
    sin_b = cs_pool.tile([128, QBLK], F32, name="sin_0", tag="sin")
    nc.sync.dma_start(out=sin_b, in_=sinS[:, 0:QBLK])
    cs_tiles[0] = (cos_b, sin_b)

    wp_sb = consts.tile([128, 4, C], F32R, name="wp_sb")  # loaded after A0
    mD_sb = consts.tile([128, MASKW], F32R, name="mD_sb")
    nc.sync.dma_start(out=mD_sb, in_=maskD[:, :])
    mT_sb = consts.tile([128, MASKW], F32R, name="mT_sb")
    nc.sync.dma_start(out=mT_sb, in_=maskTl[:, :])
    # all-ones views carved out of the diag mask (f32r, DMA-fed):
    ones_col = mD_sb[:, MASKW - 1:MASKW]     # [128, 1]
    ones_row = mD_sb[0:1, 384:512]           # [1, 128]
    ident = consts.tile([128, 128], F32, name="ident")
    make_identity(nc, ident)
    # warm the ACT exp/tanh table set during the startup DMAs (first real
    # tanh would otherwise pay the ~2.7us ACT_TABLE_LOAD mid-pipeline)
    warmup = consts.tile([1, 1], F32, name="warmup")
    nc.scalar.activation(warmup, ident[0:1, 0:1],
                         mybir.ActivationFunctionType.Tanh)

    # persistent activations (written per block, sub-tile deps handle reuse)
    kT_sb = consts.tile([128, T], F32R, name="kT_sb")          # roped k, [d, t]
    v_sb = consts.tile([128, NKT, 128], F32R, name="v_sb")     # [t128, kt, d]
    qT_sb = consts.tile([128, 4, QBLK], F32R, name="qT_sb")    # roped q, [d, h, i]
    y_tiles = {}  # qt -> [128, 4, QBLK] tile, bufs=2 across blocks

    def emit_loads(qt):
        t0 = qt * QBLK
        for k in range(NKT):
            x_t = xt_pool.tile([128, QBLK], F32R, name=f"x_{qt}_{k}", tag="xt")
            nc.sync.dma_start(out=x_t, in_=xT[k * 128:(k + 1) * 128, t0:t0 + QBLK])
            x_tiles[(qt, k)] = x_t
        cos_b = cs_pool.tile([128, QBLK], F32, name=f"cos_{qt}", tag="cos")
        nc.sync.dma_start(out=cos_b, in_=cosT[:, t0:t0 + QBLK])
        sin_b = cs_pool.tile([128, QBLK], F32, name=f"sin_{qt}", tag="sin")
        nc.sync.dma_start(out=sin_b, in_=sinS[:, t0:t0 + QBLK])
        cs_tiles[qt] = (cos_b, sin_b)

    def emit_qkv_mtile(qt, m):
        t0 = qt * QBLK
        cos_b, sin_b = cs_tiles[qt]
        psA = ps.tile([128, QBLK], F32, name=f"psA_{qt}_{m}", tag="psA")
        for k in range(NKT):
            nc.tensor.matmul(
                psA,
                wq_sb[:, k, m * 128:(m + 1) * 128],
                x_tiles[(qt, k)],
                start=(k == 0),
                stop=(k == NKT - 1),
            )
        if m < 5:
            # RoPE: dest = x*cos + rot(x)*sin_signed ; rot via DMA half-swap
            x_sb = rope_pool.tile([128, QBLK], F32, name=f"xsb_{qt}_{m}", tag="xsb")
            nc.scalar.copy(x_sb, psA)
            rot = rope_pool.tile([128, QBLK], F32, name=f"rot_{qt}_{m}", tag="rot")
            nc.gpsimd.dma_start(out=rot[0:64, :], in_=x_sb[64:128, :])
            nc.gpsimd.dma_start(out=rot[64:128, :], in_=x_sb[0:64, :])
            dest = qT_sb[:, m, :] if m < 4 else kT_sb[:, t0:t0 + QBLK]
            nc.vector.tensor_mul(x_sb, x_sb, cos_b)
            nc.vector.tensor_mul(rot, rot, sin_b)
            nc.vector.tensor_add(dest, x_sb, rot)
        else:
            # v: transpose [d, t] -> [t, d] tiles via PE
            vt_sb = rope_pool.tile([128, QBLK], F32, name=f"vt_{qt}", tag="xsb")
            nc.scalar.copy(vt_sb, psA)
            for i in range(4):
                psT = ps.tile([128, 128], F32, name=f"psT_{qt}_{i}", tag="psS")
                nc.tensor.transpose(psT, vt_sb[:, i * 128:(i + 1) * 128], ident)
                nc.vector.tensor_copy(v_sb[:, qt * 4 + i, :], psT)

    def head_mms(qt, h, wl):
        """Scores/tanh-exp/mask/pv/rowsum matmul stream for one head.
        Scores are emitted one j-tile ahead of the pv/rowsum consumers so the
        PE never sits waiting on the ACT tanh+exp latency of the current tile.
        Returns the PSUM tiles needed by the normalize stages."""
        psY = ps.tile([128, QBLK], F32, name=f"psY_{qt}_{h}", tag="psY", bufs=3)
        psR = ps.tile([1, QBLK], F32, name=f"psR_{qt}_{h}", tag="psR", bufs=1)

        def emit_scores(idx):
            kt, mk = wl[idx]
            psS = ps.tile([128, QBLK], F32, name=f"psS_{qt}_{h}_{kt}", tag="psS")
            nc.tensor.matmul(
                psS, kT_sb[:, kt * 128:(kt + 1) * 128], qT_sb[:, h, :],
                start=True, stop=True,
            )
            p_t = p_pool.tile([128, QBLK], F32R, name=f"p_{qt}_{h}_{kt}", tag="p")
            nc.scalar.activation(
                p_t, psS, mybir.ActivationFunctionType.Tanh, scale=scale1
            )
            nc.scalar.activation(
                p_t, p_t, mybir.ActivationFunctionType.Exp, scale=SOFTCAP
            )
            if mk is not None:
                msk = mD_sb if mk[0] == "D" else mT_sb
                r = mk[1]
                nc.vector.tensor_mul(p_t, p_t, msk[:, 384 - 128 * r: 896 - 128 * r])
            return p_t

        pts = {0: emit_scores(0)}
        for idx, (kt, mk) in enumerate(wl):
            if idx + 1 < len(wl):
                pts[idx + 1] = emit_scores(idx + 1)
            p_t = pts.pop(idx)
            first, last = idx == 0, idx == len(wl) - 1
            nc.tensor.matmul(
                psY, v_sb[:, kt, :], p_t,
                start=first, stop=last, skip_group_check=True,
            )
            nc.tensor.matmul(
                psR, ones_col, p_t,
                start=first, stop=last, skip_group_check=True,
            )
        return psY, psR

    def norm_head(qt, h, psY, psR):
        """Free both PSUM accumulators fast with copies, then run the
        reciprocal + partition-broadcast + multiply entirely on DVE/DMA —
        the PE never participates. recip runs on a [128,4] reshape (DVE
        recip is ~6 cyc/elem/lane; [1,512] would serialize 3.3us)."""
        rs = r_pool.tile([1, QBLK], F32, name=f"rs_{qt}_{h}", tag="rs")
        nc.vector.tensor_copy(rs, psR)
        yun = r_pool.tile([128, QBLK], F32, name=f"yun_{qt}_{h}", tag="yun")
        nc.vector.tensor_copy(yun, psY)
        rs128 = r_pool.tile([128, 4], F32, name=f"rs128_{qt}_{h}", tag="rs128")
        in_lin = bass.AP(tensor=rs.tensor, offset=rs.offset,
                         ap=[list(rs.ap[0]), [1, QBLK]])
        nc.gpsimd.dma_start(out=rs128, in_=in_lin)
        rr128 = r_pool.tile([128, 4], F32, name=f"rr128_{qt}_{h}", tag="rr128")
        nc.vector.reciprocal(rr128, rs128)
        # bounce through DRAM to broadcast across partitions (stride-0 DRAM
        # read on the way back — the standard partition-broadcast pattern)
        slot = rscratch[qt * 4 + h, :]
        nc.gpsimd.dma_start(out=slot, in_=rr128)
        rrf = r_pool.tile([128, QBLK], F32, name=f"rrf_{qt}_{h}", tag="rrf")
        bcast = bass.AP(tensor=slot.tensor, offset=slot.offset,
                        ap=[[0, 128], list(slot.ap[-1])])
        nc.gpsimd.dma_start(out=rrf, in_=bcast)
        nc.vector.tensor_mul(y_tiles[qt][:, h, :], yun, rrf)

    def emit_proj_chunk(qt, mt):
        t0 = qt * QBLK
        yt = y_tiles[qt]
        for cn in range(4):
            psP = ps.tile([128, 512], F32, name=f"psP_{qt}_{mt}_{cn}", tag="psA")
            for kh in range(4):
                nc.tensor.matmul(
                    psP,
                    yt[:, kh, mt * 128:(mt + 1) * 128],
                    wp_sb[:, kh, cn * 512:(cn + 1) * 512],
                    start=(kh == 0),
                    stop=(kh == 3),
                )
            o_t = o_pool.tile([128, 512], F32, name=f"o_{qt}_{mt}_{cn}", tag="o")
            nc.vector.tensor_copy(o_t, psP)
            nc.sync.dma_start(
                out=out[t0 + mt * 128: t0 + (mt + 1) * 128,
                        cn * 512:(cn + 1) * 512],
                in_=o_t,
            )

    # ---- interleaved pipeline ----
    # Per block qt: heads h=0..3 of A(qt) alternate with Q(qt+1) m-tiles
    # and P(qt-1) proj chunks; k/v m-tiles of Q(qt+1) follow the heads.
    for m in range(6):
        emit_qkv_mtile(0, m)
    for qt in range(NQB):
        if qt + 1 < NQB:
            emit_loads(qt + 1)
        wl = _window(qt)
        y_tiles[qt] = consts.tile([128, 4, QBLK], F32R,
                                  name=f"yT_{qt}", tag="yT", bufs=2)
        for h in range(4):
            psY, psR = head_mms(qt, h, wl)
            norm_head(qt, h, psY, psR)
            if qt + 1 < NQB:
                emit_qkv_mtile(qt + 1, (4, 5, 0, 1)[h])
            if qt >= 1:
                emit_proj_chunk(qt - 1, h)
        if qt == 0:
            for k in range(4):
                nc.sync.dma_start(out=wp_sb[:, k, :],
                                  in_=wprojT[k * 128:(k + 1) * 128, :])
        if qt + 1 < NQB:
            emit_qkv_mtile(qt + 1, 2)
            emit_qkv_mtile(qt + 1, 3)
    for mt in range(4):
        emit_proj_chunk(NQB - 1, mt)

_NC_CACHE = {}


def _build_nc():
    if "nc" not in _NC_CACHE:
        from contextlib import ExitStack

        from concourse import bacc

        nc = bacc.Bacc()
        with tile.TileContext(nc) as tc, ExitStack() as ctx:
            _emit(tc, ctx)
        nc.compile()
        _NC_CACHE["nc"] = nc
    return _NC_CACHE["nc"]


def _host_inputs(x, cos, sin, Wqkv, Wproj):
    """Build the 8 per-core input maps (sharding + layout transforms)."""
    cosT = np.ascontiguousarray(cos.T)                       # [128, T]
    sinT = sin.T
    sinS = np.concatenate([-sinT[:64], sinT[64:]], axis=0)   # sign-folded rotate-half
    sinS = np.ascontiguousarray(sinS)
    lj = np.arange(128)[:, None]
    xcol = np.arange(MASKW)[None, :] - 384                   # x = li - 128r
    maskD = (xcol >= lj).astype(np.float32)                  # diag: li - 128r >= lj
    maskTl = (xcol < lj).astype(np.float32)                  # tail: li - 128r <  lj
    q_sz = N_HEAD * HEAD_SIZE

    in_maps = []
    for core in range(8):
        b, g = core // 4, core % 4
        xTb = np.ascontiguousarray(x[b].T)                   # [C, T]
        wslice = np.concatenate(
            [
                Wqkv[512 * g: 512 * (g + 1)],                 # 4 q heads
                Wqkv[q_sz + 128 * g: q_sz + 128 * (g + 1)],   # k
                Wqkv[q_sz + 512 + 128 * g: q_sz + 512 + 128 * (g + 1)],  # v
            ],
            axis=0,
        )                                                     # [768, C]
        wqkvT = np.ascontiguousarray(wslice.T)                # [C, 768]
        wprojT = np.ascontiguousarray(Wproj[:, 512 * g: 512 * (g + 1)].T)  # [512, C]
        in_maps.append(
            {
                "xT": xTb,
                "wqkvT": wqkvT,
                "wprojT": wprojT,
                "cosT": cosT,
                "sinS": sinS,
                "maskD": maskD,
                "maskTl": maskTl,
            }
        )
    return in_maps


def kernel(x, cos, sin, Wqkv, Wproj, trace=False, tmpdir=None):
    x = np.asarray(x, dtype=np.float32)
    cos = np.asarray(cos, dtype=np.float32)
    sin = np.asarray(sin, dtype=np.float32)
    Wqkv = np.asarray(Wqkv, dtype=np.float32)
    Wproj = np.asarray(Wproj, dtype=np.float32)

    nc = _build_nc()
    in_maps = _host_inputs(x, cos, sin, Wqkv, Wproj)
    res = run_bass_kernel_spmd(nc, in_maps, list(range(8)), trace=trace, tmpdir=tmpdir)
    parts = [res.results[i]["out"] for i in range(8)]
    out = np.empty((B, T, C), dtype=np.float32)
    out[0] = parts[0] + parts[1] + parts[2] + parts[3]
    out[1] = parts[4] + parts[5] + parts[6] + parts[7]
    if trace:
        kernel.last_exec_time_ns = res.exec_time_ns
        kernel.last_results = res
    return out


# revision 27
# speedup vs baseline: 1.0839x; 1.0375x over previous
"""Sliding-window causal self-attention (GQA + RoPE + tanh softcap) on 8 trn2 cores.

Sharding: core i = (b, g) with b = i // 4, g = i % 4.
Each core computes, for its batch b and kv-group g (4 q heads, 1 kv head):
    qkv projection (o-slice), RoPE, sliding-window attention, and the proj
    contribution of its o-slice:  out_partial[t, c] = sum_{o in slice} y[t,o] Wproj[c,o].
The host sums the 4 partials per batch (the "all-reduce after proj" done host-side).

All matmuls run as fp32r (full PE rate at N=512 when warm). Layouts avoid
on-device transposes except v (PE-transpose via identity):
    xT      [C, T]   (host-transposed x[b])
    wqkvT   [C, 768] (host-transposed o-slice of Wqkv; o order: q0..q3, k, v)
    wprojT  [512, C] (host-transposed o-slice columns of Wproj)
    qT/kT   [d, t]   from  wT.T @ xT  (d on partitions -> scoresT = kT_tile.T @ qT)
    scoresT [j, i]   j (keys) on partitions, i (queries) on free axis
    P = exp(softcap(scores)) stays [j, i]; yT = v_tile.T @ P accumulates [d, i]
    rowsum via ones-matmul [1, i]; normalize = PE-broadcast recip + TT mult.

Block pipeline (emission order): L0 Q0 L1 A0 Q1 P0 L2 A1 Q2 P1 L3 A2 Q3 P2 A3 P3
so the PE always has qkv/proj matmul work while attention tails (rowsum
reciprocal chains) drain on DVE/DMA. Each head's normalize is split:
  stageA (DVE copy of psR + reshape-DMA + recip[128,4] + DMA back) emitted
  right after the head's matmuls; stageC (psB broadcast MM + rrf copy + yT
  mult) deferred past the next head's matmuls so the PE never waits on it.
"""

import math

import numpy as np

import concourse.bass as bass
import concourse.mybir as mybir
import concourse.tile as tile
from concourse.bass_utils import run_bass_kernel_spmd
from concourse.masks import make_identity

B, T, C = 2, 2048, 2048
N_HEAD, N_GROUPS, HEAD_SIZE = 16, 4, 128
SW = 1024
SOFTCAP = 50.0
QBLK = 512
NQB = T // QBLK          # 4 q-blocks
NKT = T // 128           # 16 key tiles
O_SLICE = 768            # 4 q heads + k + v  (128 each)
F32 = mybir.dt.float32
F32R = mybir.dt.float32r
MASKW = 896              # staircase mask width: x = li - 128r, x in [-384, 512)


def _window(qt):
    """Key-tile list for q-block qt: (kt, mask) with mask None | ('D', r) | ('T', r)."""
    wl = []
    if qt >= 2:
        for r in range(4):
            wl.append((4 * qt - 8 + r, ("T", r)))
    for kt in range(max(0, 4 * qt - 4), 4 * qt):
        wl.append((kt, None))
    for r in range(4):
        wl.append((4 * qt + r, ("D", r)))
    return wl


def _emit(tc, ctx):
    nc = tc.nc
    xT = nc.declare_dram_parameter("xT", [C, T], F32R, isOutput=False)
    wqkvT = nc.declare_dram_parameter("wqkvT", [C, O_SLICE], F32R, isOutput=False)
    wprojT = nc.declare_dram_parameter("wprojT", [512, C], F32R, isOutput=False)
    cosT = nc.declare_dram_parameter("cosT", [HEAD_SIZE, T], F32, isOutput=False)
    sinS = nc.declare_dram_parameter("sinS", [HEAD_SIZE, T], F32, isOutput=False)
    maskD = nc.declare_dram_parameter("maskD", [128, MASKW], F32R, isOutput=False)
    maskTl = nc.declare_dram_parameter("maskTl", [128, MASKW], F32R, isOutput=False)
    out = nc.declare_dram_parameter("out", [T, C], F32, isOutput=True)
    rscratch = nc.dram_tensor("rscratch", [NQB * 4, QBLK], F32)

    scale1 = 1.0 / (SOFTCAP * math.sqrt(HEAD_SIZE))

    consts = ctx.enter_context(tc.tile_pool(name="consts", bufs=1))
    xt_pool = ctx.enter_context(tc.tile_pool(name="xt", bufs=16))
    cs_pool = ctx.enter_context(tc.tile_pool(name="cs", bufs=2))
    rope_pool = ctx.enter_context(tc.tile_pool(name="rope", bufs=3))
    p_pool = ctx.enter_context(tc.tile_pool(name="pp", bufs=5))
    o_pool = ctx.enter_context(tc.tile_pool(name="op", bufs=3))
    r_pool = ctx.enter_context(tc.tile_pool(name="rp", bufs=2))
    ps = ctx.enter_context(tc.tile_pool(name="ps", space="PSUM", bufs=2))

    # ---- resident constants (x block 0 interleaved for fast PE start) ----
    wq_sb = consts.tile([128, NKT, O_SLICE], F32R, name="wq_sb")
    x_tiles = {}  # (qt, k) -> tile
    for k in range(NKT):
        nc.sync.dma_start(out=wq_sb[:, k, :], in_=wqkvT[k * 128:(k + 1) * 128, :])
        x_t = xt_pool.tile([128, QBLK], F32R, name=f"x_0_{k}", tag="xt")
        nc.sync.dma_start(out=x_t, in_=xT[k * 128:(k + 1) * 128, 0:QBLK])
        x_tiles[(0, k)] = x_t
    cs_tiles = {}
    cos_b = cs_pool.tile([128, QBLK], F32, name="cos_0", tag="cos")
    nc.sync.dma_start(out=cos_b, in_=cosT[:, 0:QBLK])
    sin_b = cs_pool.tile([128, QBLK], F32, name="sin_0", tag="sin")
    nc.sync.dma_start(out=sin_b, in_=sinS[:, 0:QBLK])
    cs_tiles[0] = (cos_b, sin_b)

    wp_sb = consts.tile([128, 4, C], F32R, name="wp_sb")  # loaded after A0
    mD_sb = consts.tile([128, MASKW], F32R, name="mD_sb")
    nc.sync.dma_start(out=mD_sb, in_=maskD[:, :])
    mT_sb = consts.tile([128, MASKW], F32R, name="mT_sb")
    nc.sync.dma_start(out=mT_sb, in_=maskTl[:, :])
    # all-ones views carved out of the diag mask (f32r, DMA-fed):
    ones_col = mD_sb[:, MASKW - 1:MASKW]     # [128, 1]
    ones_row = mD_sb[0:1, 384:512]           # [1, 128]
    ident = consts.tile([128, 128], F32, name="ident")
    make_identity(nc, ident)
    # warm the ACT exp/tanh table set during the startup DMAs (first real
    # tanh would otherwise pay the ~2.7us ACT_TABLE_LOAD mid-pipeline)
    warmup = consts.tile([1, 1], F32, name="warmup")
    nc.scalar.activation(warmup, ident[0:1, 0:1],
                         mybir.ActivationFunctionType.Tanh)

    # persistent activations (written per block, sub-tile deps handle reuse)
    kT_sb = consts.tile([128, T], F32R, name="kT_sb")          # roped k, [d, t]
    v_sb = consts.tile([128, NKT, 128], F32R, name="v_sb")     # [t128, kt, d]
    qT_sb = consts.tile([128, 4, QBLK], F32R, name="qT_sb")    # roped q, [d, h, i]
    y_tiles = {}  # qt -> [128, 4, QBLK] tile, bufs=2 across blocks

    def emit_loads(qt):
        t0 = qt * QBLK
        for k in range(NKT):
            x_t = xt_pool.tile([128, QBLK], F32R, name=f"x_{qt}_{k}", tag="xt")
            nc.sync.dma_start(out=x_t, in_=xT[k * 128:(k + 1) * 128, t0:t0 + QBLK])
            x_tiles[(qt, k)] = x_t
        cos_b = cs_pool.tile([128, QBLK], F32, name=f"cos_{qt}", tag="cos")
        nc.sync.dma_start(out=cos_b, in_=cosT[:, t0:t0 + QBLK])
        sin_b = cs_pool.tile([128, QBLK], F32, name=f"sin_{qt}", tag="sin")
        nc.sync.dma_start(out=sin_b, in_=sinS[:, t0:t0 + QBLK])
        cs_tiles[qt] = (cos_b, sin_b)

    def emit_rope(qt, m, psA):
        t0 = qt * QBLK
        cos_b, sin_b = cs_tiles[qt]
        if m < 5:
            # RoPE: dest = x*cos + rot(x)*sin_signed ; rot via DMA half-swap
            x_sb = rope_pool.tile([128, QBLK], F32, name=f"xsb_{qt}_{m}", tag="xsb")
            nc.scalar.copy(x_sb, psA)
            rot = rope_pool.tile([128, QBLK], F32, name=f"rot_{qt}_{m}", tag="rot")
            nc.gpsimd.dma_start(out=rot[0:64, :], in_=x_sb[64:128, :])
            nc.gpsimd.dma_start(out=rot[64:128, :], in_=x_sb[0:64, :])
            dest = qT_sb[:, m, :] if m < 4 else kT_sb[:, t0:t0 + QBLK]
            nc.vector.tensor_mul(x_sb, x_sb, cos_b)
            nc.vector.tensor_mul(rot, rot, sin_b)
            nc.vector.tensor_add(dest, x_sb, rot)
        else:
            # v: transpose [d, t] -> [t, d] tiles via PE
            vt_sb = rope_pool.tile([128, QBLK], F32, name=f"vt_{qt}", tag="xsb")
            nc.scalar.copy(vt_sb, psA)
            for i in range(4):
                psT = ps.tile([128, 128], F32, name=f"psT_{qt}_{i}", tag="psS")
                nc.tensor.transpose(psT, vt_sb[:, i * 128:(i + 1) * 128], ident)
                nc.vector.tensor_copy(v_sb[:, qt * 4 + i, :], psT)

    def qkv_units(qt, m):
        """Fill units for one qkv m-tile: 8 x 2-matmul chunks + rope drain.
        Unit cost estimates are in ~us of PE time for the pop budget."""
        hold = {}

        def mk(i):
            def emit():
                if i == 0:
                    hold["psA"] = ps.tile([128, QBLK], F32,
                                          name=f"psA_{qt}_{m}", tag="psA")
                psA = hold["psA"]
                for k in (2 * i, 2 * i + 1):
                    nc.tensor.matmul(
                        psA,
                        wq_sb[:, k, m * 128:(m + 1) * 128],
                        x_tiles[(qt, k)],
                        start=(k == 0),
                        stop=(k == NKT - 1),
                    )
            return emit

        units = [(0.46, mk(i)) for i in range(8)]
        units.append((0.1, lambda: emit_rope(qt, m, hold["psA"])))
        return units

    def emit_qkv_mtile(qt, m):
        for _, emit in qkv_units(qt, m):
            emit()

    from collections import deque

    fill_q = deque()

    def pop_fill(budget):
        """Emit queued qkv/proj matmul units worth ~budget us of PE time —
        keeps the PE fed while the attention stream waits on ACT latency."""
        spent = 0.0
        while fill_q and spent < budget:
            cost, emit = fill_q.popleft()
            emit()
            spent += cost

    def head_mms(qt, h, wl):
        """Scores/tanh-exp/mask/pv/rowsum matmul stream for one head.
        Scores are emitted one j-tile ahead of the pv/rowsum consumers so the
        PE never sits waiting on the ACT tanh+exp latency of the current tile.
        Returns the PSUM tiles needed by the normalize stages."""
        psY = ps.tile([128, QBLK], F32, name=f"psY_{qt}_{h}", tag="psY", bufs=3)
        psR = ps.tile([1, QBLK], F32, name=f"psR_{qt}_{h}", tag="psR", bufs=1)

        def emit_scores(idx):
            kt, mk = wl[idx]
            psS = ps.tile([128, QBLK], F32, name=f"psS_{qt}_{h}_{kt}", tag="psS")
            nc.tensor.matmul(
                psS, kT_sb[:, kt * 128:(kt + 1) * 128], qT_sb[:, h, :],
                start=True, stop=True,
            )
            p_t = p_pool.tile([128, QBLK], F32R, name=f"p_{qt}_{h}_{kt}", tag="p")
            nc.scalar.activation(
                p_t, psS, mybir.ActivationFunctionType.Tanh, scale=scale1
            )
            nc.scalar.activation(
                p_t, p_t, mybir.ActivationFunctionType.Exp, scale=SOFTCAP
            )
            if mk is not None:
                msk = mD_sb if mk[0] == "D" else mT_sb
                r = mk[1]
                nc.vector.tensor_mul(p_t, p_t, msk[:, 384 - 128 * r: 896 - 128 * r])
            return p_t

        pts = {0: emit_scores(0)}
        for idx, (kt, mk) in enumerate(wl):
            if idx + 1 < len(wl):
                pts[idx + 1] = emit_scores(idx + 1)
            pop_fill(0.85)
            p_t = pts.pop(idx)
            first, last = idx == 0, idx == len(wl) - 1
            nc.tensor.matmul(
                psY, v_sb[:, kt, :], p_t,
                start=first, stop=last, skip_group_check=True,
            )
            nc.tensor.matmul(
                psR, ones_col, p_t,
                start=first, stop=last, skip_group_check=True,
            )
        return psY, psR

    def norm_head(qt, h, psY, psR):
        """Free both PSUM accumulators fast with copies, then run the
        reciprocal + partition-broadcast + multiply entirely on DVE/DMA —
        the PE never participates. recip runs on a [128,4] reshape (DVE
        recip is ~6 cyc/elem/lane; [1,512] would serialize 3.3us)."""
        rs = r_pool.tile([1, QBLK], F32, name=f"rs_{qt}_{h}", tag="rs")
        nc.vector.tensor_copy(rs, psR)
        yun = r_pool.tile([128, QBLK], F32, name=f"yun_{qt}_{h}", tag="yun")
        nc.vector.tensor_copy(yun, psY)
        rs128 = r_pool.tile([128, 4], F32, name=f"rs128_{qt}_{h}", tag="rs128")
        in_lin = bass.AP(tensor=rs.tensor, offset=rs.offset,
                         ap=[list(rs.ap[0]), [1, QBLK]])
        nc.gpsimd.dma_start(out=rs128, in_=in_lin)
        rr128 = r_pool.tile([128, 4], F32, name=f"rr128_{qt}_{h}", tag="rr128")
        nc.vector.reciprocal(rr128, rs128)
        # bounce through DRAM to broadcast across partitions (stride-0 DRAM
        # read on the way back — the standard partition-broadcast pattern)
        slot = rscratch[qt * 4 + h, :]
        nc.gpsimd.dma_start(out=slot, in_=rr128)
        rrf = r_pool.tile([128, QBLK], F32, name=f"rrf_{qt}_{h}", tag="rrf")
        bcast = bass.AP(tensor=slot.tensor, offset=slot.offset,
                        ap=[[0, 128], list(slot.ap[-1])])
        nc.gpsimd.dma_start(out=rrf, in_=bcast)
        nc.vector.tensor_mul(y_tiles[qt][:, h, :], yun, rrf)

    def proj_units(qt, mt):
        t0 = qt * QBLK

        def mk(cn):
            def emit():
                psP = ps.tile([128, 512], F32,
                              name=f"psP_{qt}_{mt}_{cn}", tag="psA")
                yt = y_tiles[qt]
                for kh in range(4):
                    nc.tensor.matmul(
                        psP,
                        yt[:, kh, mt * 128:(mt + 1) * 128],
                        wp_sb[:, kh, cn * 512:(cn + 1) * 512],
                        start=(kh == 0),
                        stop=(kh == 3),
                    )
                o_t = o_pool.tile([128, 512], F32,
                                  name=f"o_{qt}_{mt}_{cn}", tag="o")
                nc.vector.tensor_copy(o_t, psP)
                nc.sync.dma_start(
                    out=out[t0 + mt * 128: t0 + (mt + 1) * 128,
                            cn * 512:(cn + 1) * 512],
                    in_=o_t,
                )
            return emit

        return [(0.9, mk(cn)) for cn in range(4)]

    def emit_proj_chunk(qt, mt):
        for _, emit in proj_units(qt, mt):
            emit()

    # ---- interleaved pipeline with fine-grained fills ----
    # qkv m-tiles of block qt+1 (kv first) and proj chunks of block qt-1 are
    # queued as small matmul units and drained INSIDE the attention jt loops,
    # so the PE stays busy while ACT works through tanh/exp and ACT stays
    # busy (next scores arrive promptly) while the PE runs fills.
    for m in range(6):
        emit_qkv_mtile(0, m)
    for qt in range(NQB):
        if qt + 1 < NQB:
            emit_loads(qt + 1)
        wl = _window(qt)
        y_tiles[qt] = consts.tile([128, 4, QBLK], F32R,
                                  name=f"yT_{qt}", tag="yT", bufs=2)
        for h in range(4):
            if qt + 1 < NQB:
                fill_q.extend(qkv_units(qt + 1, (4, 5, 0, 1)[h]))
            if qt >= 1:
                fill_q.extend(proj_units(qt - 1, h))
            psY, psR = head_mms(qt, h, wl)
            norm_head(qt, h, psY, psR)
        if qt == 0:
            for k in range(4):
                nc.sync.dma_start(out=wp_sb[:, k, :],
                                  in_=wprojT[k * 128:(k + 1) * 128, :])
        if qt + 1 < NQB:
            fill_q.extend(qkv_units(qt + 1, 2))
            fill_q.extend(qkv_units(qt + 1, 3))
    while fill_q:
        fill_q.popleft()[1]()
    for mt in range(4):
        emit_proj_chunk(NQB - 1, mt)

_NC_CACHE = {}


def _build_nc():
    if "nc" not in _NC_CACHE:
        from contextlib import ExitStack

        from concourse import bacc

        nc = bacc.Bacc()
        with tile.TileContext(nc) as tc, ExitStack() as ctx:
            _emit(tc, ctx)
        nc.compile()
        _NC_CACHE["nc"] = nc
    return _NC_CACHE["nc"]


def _host_inputs(x, cos, sin, Wqkv, Wproj):
    """Build the 8 per-core input maps (sharding + layout transforms)."""
    cosT = np.ascontiguousarray(cos.T)                       # [128, T]
    sinT = sin.T
    sinS = np.concatenate([-sinT[:64], sinT[64:]], axis=0)   # sign-folded rotate-half
    sinS = np.ascontiguousarray(sinS)
    lj = np.arange(128)[:, None]
    xcol = np.arange(MASKW)[None, :] - 384                   # x = li - 128r
    maskD = (xcol >= lj).astype(np.float32)                  # diag: li - 128r >= lj
    maskTl = (xcol < lj).astype(np.float32)                  # tail: li - 128r <  lj
    q_sz = N_HEAD * HEAD_SIZE

    in_maps = []
    for core in range(8):
        b, g = core // 4, core % 4
        xTb = np.ascontiguousarray(x[b].T)                   # [C, T]
        wslice = np.concatenate(
            [
                Wqkv[512 * g: 512 * (g + 1)],                 # 4 q heads
                Wqkv[q_sz + 128 * g: q_sz + 128 * (g + 1)],   # k
                Wqkv[q_sz + 512 + 128 * g: q_sz + 512 + 128 * (g + 1)],  # v
            ],
            axis=0,
        )                                                     # [768, C]
        wqkvT = np.ascontiguousarray(wslice.T)                # [C, 768]
        wprojT = np.ascontiguousarray(Wproj[:, 512 * g: 512 * (g + 1)].T)  # [512, C]
        in_maps.append(
            {
                "xT": xTb,
                "wqkvT": wqkvT,
                "wprojT": wprojT,
                "cosT": cosT,
                "sinS": sinS,
                "maskD": maskD,
                "maskTl": maskTl,
            }
        )
    return in_maps


def kernel(x, cos, sin, Wqkv, Wproj, trace=False, tmpdir=None):
    x = np.asarray(x, dtype=np.float32)
    cos = np.asarray(cos, dtype=np.float32)
    sin = np.asarray(sin, dtype=np.float32)
    Wqkv = np.asarray(Wqkv, dtype=np.float32)
    Wproj = np.asarray(Wproj, dtype=np.float32)

    nc = _build_nc()
    in_maps = _host_inputs(x, cos, sin, Wqkv, Wproj)
    res = run_bass_kernel_spmd(nc, in_maps, list(range(8)), trace=trace, tmpdir=tmpdir)
    parts = [res.results[i]["out"] for i in range(8)]
    out = np.empty((B, T, C), dtype=np.float32)
    out[0] = parts[0] + parts[1] + parts[2] + parts[3]
    out[1] = parts[4] + parts[5] + parts[6] + parts[7]
    if trace:
        kernel.last_exec_time_ns = res.exec_time_ns
        kernel.last_results = res
    return out
